# revision 9
# baseline (speedup 1.0000x reference)
"""Sliding-window causal GQA attention block (QKV proj + RoPE + SDPA + out proj)
on 8 Trainium2 NeuronCores.

Sharding: 8 cores = 2 batches x 4 sequence chunks of 512 tokens. Each core
computes the attention-block output for its (batch, seq-chunk):
  - K/V projection for its OWN 512 tokens only; the 512-token halo needed by
    the sliding window is exchanged between cores with an AllGather over each
    batch's 4-core group plus a one-hot select (chunk-0 cores select zeros).
  - Q projection for its 512 queries (all 16 heads) in transposed [d, s]
    layout; RoPE via rotate-half permutation matmul + element-wise mul/add.
  - attention runs on blocks of (kv-group, 128-query sub-chunk) with all 4
    query heads of the group sharing the 512 matmul columns; each block needs
    exactly 5 key tiles of 128 (vs 6 for 256-query blocks).
  - softmax denominators: the 5 masked-exp tiles are summed on the vector
    engine, then a single ones-vector matmul per block reduces over keys.
  - out-projection computed transposed (y^T = wo^T-tiles @ o^T); host
    transposes each core's bf16 slab back and casts to f32.

Weights/x are pre-packed on the host into per-partition-contiguous layouts so
every DMA moves 4-16 KB per partition line. Matmul operands are bf16, all
accumulation fp32 in PSUM.
"""
import numpy as np

import concourse.bacc as bacc
import concourse.mybir as mybir
import concourse.tile as tile
from concourse.bass_utils import run_bass_kernel_spmd

# Problem constants (hardcoded per contract)
B, S, E = 2, 2048, 2048
H, KV, D = 16, 4, 128
WIN = 512
THETA = 1e6
NCORES = 8
CH = 512          # seq chunk per core
SW = 1024         # K/V window per core (halo 512 + own 512)
P = 128
ECH = E // P      # 16 contraction chunks
F32 = mybir.dt.float32
BF16 = mybir.dt.bfloat16
SCALE = 1.0 / float(np.sqrt(np.float32(D)))

USE_HALO_EXCHANGE = True

_CACHE = {}


def _build():
    nc = bacc.Bacc("TRN2", target_bir_lowering=False, debug=False,
                   num_devices=NCORES)

    # host-prepacked inputs: per-partition-contiguous layouts
    xt_own = nc.dram_tensor("xt_own", [P, ECH * CH], BF16, kind="ExternalInput")
    wq_all = nc.dram_tensor("wq_all", [H, P, ECH * P], BF16, kind="ExternalInput")
    wk_all = nc.dram_tensor("wk_all", [KV, P, ECH * P], BF16, kind="ExternalInput")
    wv_all = nc.dram_tensor("wv_all", [P, ECH * KV * D], BF16, kind="ExternalInput")
    wo_all = nc.dram_tensor("wo_all", [ECH, P, H * P], BF16, kind="ExternalInput")
    cosc = nc.dram_tensor("cosc", [P, CH], F32, kind="ExternalInput")
    sinc = nc.dram_tensor("sinc", [P, CH], F32, kind="ExternalInput")
    masks = nc.dram_tensor("masks", [20, P, CH], BF16, kind="ExternalInput")
    perm = nc.dram_tensor("perm", [P, P], BF16, kind="ExternalInput")
    ones = nc.dram_tensor("ones", [1, P], BF16, kind="ExternalInput")
    yt = nc.dram_tensor("yt", [E, CH], BF16, kind="ExternalOutput")
    if USE_HALO_EXCHANGE:
        sel = nc.dram_tensor("sel", [P, 4], F32, kind="ExternalInput")
        cc_in = nc.dram_tensor("cc_in", [P, 2 * KV * CH], BF16, kind="Internal")
        cc_out = nc.dram_tensor("cc_out", [4 * P, 2 * KV * CH], BF16,
                                kind="Internal")
    else:
        xt_halo = nc.dram_tensor("xt_halo", [P, ECH * CH], BF16,
                                 kind="ExternalInput")
        cosh = nc.dram_tensor("cosh", [P, CH], F32, kind="ExternalInput")
        sinh = nc.dram_tensor("sinh", [P, CH], F32, kind="ExternalInput")

    with tile.TileContext(nc) as tc:
        with (
            tc.tile_pool(name="res", bufs=1) as res,       # resident tensors
            tc.tile_pool(name="wvp", bufs=1) as wvp,       # resident wv
            tc.tile_pool(name="hlp", bufs=2) as hlp,       # halo block stream
            tc.tile_pool(name="wkp", bufs=2) as wkp,       # streamed wk tiles
            tc.tile_pool(name="wqp", bufs=2) as wqp,       # streamed wq tiles
            tc.tile_pool(name="wop", bufs=2) as wop,       # streamed wo tiles
            tc.tile_pool(name="tmp", bufs=3) as tmp,       # transient compute
            tc.tile_pool(name="acc", bufs=2) as accp,      # select/pt-sum accums
            tc.tile_pool(name="pj", bufs=2, space="PSUM") as pj,
            tc.tile_pool(name="ps1", bufs=2, space="PSUM") as ps1,  # scores
            tc.tile_pool(name="ps2", bufs=2, space="PSUM") as ps2,  # av
            tc.tile_pool(name="psd", bufs=2, space="PSUM") as psd,  # denom
        ):
            # ------------ x own chunk (sync queue, pieces for early start) ---
            x_own = res.tile([P, ECH, CH], BF16, tag="xown")
            xt3 = xt_own.ap().rearrange("p (eo s) -> p eo s", eo=ECH)
            for lo, hi in ((0, 2), (2, 4), (4, 8), (8, 12), (12, 16)):
                nc.sync.dma_start(x_own[:, lo:hi, :], xt3[:, lo:hi, :])

            # first K-head weights early (sync queue)
            wk_t = {}

            def load_wk(fk):
                wk_t[fk] = wkp.tile([P, ECH, P], BF16, tag="wk",
                                    name=f"wk_{fk}")
                nc.sync.dma_start(
                    wk_t[fk][:],
                    wk_all.ap().rearrange("h p c -> p h c")[:, fk, :]
                    .rearrange("p (eo c) -> p eo c", eo=ECH))

            load_wk(0)

            # ---------------- constants (gpsimd queue) ----------------------
            cos_sb = res.tile([P, CH], F32, tag="cosc")
            sin_sb = res.tile([P, CH], F32, tag="sinc")
            nc.gpsimd.dma_start(cos_sb[:], cosc.ap())
            nc.gpsimd.dma_start(sin_sb[:], sinc.ap())
            perm_sb = res.tile([P, P], BF16, tag="perm")
            nc.gpsimd.dma_start(perm_sb[:], perm.ap())
            ones_sb = res.tile([P, 1], BF16, tag="ones")
            nc.gpsimd.dma_start(ones_sb[:], ones.ap().rearrange("o p -> p o"))
            if USE_HALO_EXCHANGE:
                sel_sb = res.tile([P, 4], F32, tag="sel")
                nc.gpsimd.dma_start(sel_sb[:], sel.ap())
            else:
                x_halo = res.tile([P, ECH, CH], BF16, tag="xhalo")
                nc.gpsimd.dma_start(x_halo[:], xt_halo.ap().rearrange(
                    "p (eo s) -> p eo s", eo=ECH))
                cosh_sb = res.tile([P, CH], F32, tag="cosh")
                sinh_sb = res.tile([P, CH], F32, tag="sinh")
                nc.gpsimd.dma_start(cosh_sb[:], cosh.ap())
                nc.gpsimd.dma_start(sinh_sb[:], sinh.ap())

            # wv in the big pool, [p, e_chunk, v_cols 512] (gpsimd queue)
            wv_sb = wvp.tile([P, ECH, KV * D], BF16, tag="wv")
            nc.gpsimd.dma_start(wv_sb[:], wv_all.ap().rearrange(
                "p (eo c) -> p eo c", eo=ECH))

            # ---------------- rope helper ----------------
            def rope(dst, raw_ps, cos_t, sin_t, split4=False):
                """dst = rope(raw_ps [128, 512]) with given cos/sin [128,512]."""
                raw_sb = tmp.tile([P, CH], BF16, tag="qraw")
                nc.vector.tensor_copy(out=raw_sb[:], in_=raw_ps[:])
                rot_ps = ps1.tile([P, CH], F32, tag="sc")
                nc.tensor.matmul(rot_ps[:], perm_sb[:], raw_sb[:],
                                 start=True, stop=True)
                t1 = tmp.tile([P, CH], F32, tag="qraw")
                nc.gpsimd.tensor_mul(out=t1[:], in0=raw_sb[:], in1=cos_t[:])
                t2 = tmp.tile([P, CH], F32, tag="qraw")
                nc.vector.tensor_mul(out=t2[:], in0=rot_ps[:], in1=sin_t[:])
                if split4:
                    nc.vector.tensor_add(
                        out=dst,
                        in0=t1[:].rearrange("p (a b) -> p a b", a=4),
                        in1=t2[:].rearrange("p (a b) -> p a b", a=4))
                else:
                    nc.vector.tensor_add(out=dst, in0=t1[:], in1=t2[:])

            # ------------- K projection own chunk ([d, s] layout) ------------
            k_own = res.tile([P, KV, CH], BF16, tag="k")
            for fk in range(KV):
                if fk > 0:
                    load_wk(fk)
                k_ps = pj.tile([P, CH], F32, tag="pj")
                for e in range(ECH):
                    nc.tensor.matmul(k_ps[:], wk_t[fk][:, e, :], x_own[:, e, :],
                                     start=(e == 0), stop=(e == ECH - 1))
                rope(k_own[:, fk, :], k_ps, cos_sb, sin_sb)

            # ------------- V projection own chunk (natural [s, d] layout) ----
            v_own = res.tile([P, 4, KV * D], BF16, tag="v")
            for st in range(4):
                v_ps = pj.tile([P, KV * D], F32, tag="pj")
                for e in range(ECH):
                    nc.tensor.matmul(v_ps[:], x_own[:, e, st * P:(st + 1) * P],
                                     wv_sb[:, e, :],
                                     start=(e == 0), stop=(e == ECH - 1))
                nc.vector.tensor_copy(out=v_own[:, st, :], in_=v_ps[:])

            halo_k = res.tile([P, KV, CH], BF16, tag="hk")
            halo_v = res.tile([P, 4, KV * D], BF16, tag="hv")
            if USE_HALO_EXCHANGE:
                # pack own K/V and trigger the all-gather over the 4-core
                # batch group; the receive + select is emitted after the Q
                # projection so no engine stream stalls on the collective.
                W = KV * CH
                nc.gpsimd.dma_start(cc_in.ap()[:, 0:W],
                                    k_own[:].rearrange("p a b -> p (a b)"))
                nc.gpsimd.dma_start(cc_in.ap()[:, W:2 * W],
                                    v_own[:].rearrange("p a b -> p (a b)"))
                nc.gpsimd.collective_compute(
                    "AllGather", mybir.AluOpType.bypass,
                    ins=[cc_in.ap()], outs=[cc_out.ap()],
                    replica_groups=[[0, 1, 2, 3], [4, 5, 6, 7]])
            else:
                for fk in range(KV):
                    k_ps = pj.tile([P, CH], F32, tag="pj")
                    for e in range(ECH):
                        nc.tensor.matmul(k_ps[:], wk_t[fk][:, e, :],
                                         x_halo[:, e, :],
                                         start=(e == 0), stop=(e == ECH - 1))
                    rope(halo_k[:, fk, :], k_ps, cosh_sb, sinh_sb)
                for st in range(4):
                    v_ps = pj.tile([P, KV * D], F32, tag="pj")
                    for e in range(ECH):
                        nc.tensor.matmul(v_ps[:],
                                         x_halo[:, e, st * P:(st + 1) * P],
                                         wv_sb[:, e, :],
                                         start=(e == 0), stop=(e == ECH - 1))
                    nc.vector.tensor_copy(out=halo_v[:, st, :], in_=v_ps[:])

            def k_tile(kvb, jt):
                """lhsT [128 d, 128 keys] for window key tile jt (0..7)."""
                if jt < 4:
                    return halo_k[:, kvb, jt * P:(jt + 1) * P]
                return k_own[:, kvb, (jt - 4) * P:(jt - 3) * P]

            def v_tile(kvb, jt):
                """lhsT [128 keys, 128 d] for window key tile jt (0..7)."""
                if jt < 4:
                    return halo_v[:, jt, kvb * D:(kvb + 1) * D]
                return v_own[:, jt - 4, kvb * D:(kvb + 1) * D]

            # masks load on the scalar queue (idle until attention exps)
            mask_sb = res.tile([P, 20, CH], BF16, tag="masks")
            for mi in range(20):
                nc.scalar.dma_start(mask_sb[:, mi, :], masks.ap()[mi])

            # ------------- Q projection (transposed [d, s] layout) -----------
            # q_sb free layout [kv, qsub, h4, qcol]: block (kv, qsub) holds the
            # same 128 queries for the 4 heads of kv-group kv.
            q_sb = res.tile([P, KV, 4, 4, P], BF16, tag="q")
            wq3 = wq_all.ap().rearrange("h p c -> p h c")
            for fi in range(H):
                kvb, h4 = fi // 4, fi % 4
                wq_t = wqp.tile([P, ECH, P], BF16, tag="wq", name=f"wq_{fi}")
                nc.sync.dma_start(
                    wq_t[:],
                    wq3[:, fi, :].rearrange("p (eo c) -> p eo c", eo=ECH))
                q_ps = pj.tile([P, CH], F32, tag="pj")
                for e in range(ECH):
                    nc.tensor.matmul(q_ps[:], wq_t[:, e, :], x_own[:, e, :],
                                     start=(e == 0), stop=(e == ECH - 1))
                rope(q_sb[:, kvb, :, h4, :], q_ps, cos_sb, sin_sb, split4=True)

            if USE_HALO_EXCHANGE:
                # receive gathered K/V blocks and one-hot-select the previous
                # chunk's block into halo_k / halo_v (zeros for chunk 0)
                W = KV * CH
                for half, dst in ((0, halo_k), (1, halo_v)):
                    acc = None
                    for j in range(4):
                        hj = hlp.tile([P, W], BF16, tag="hal",
                                      name=f"hal_{half}_{j}")
                        nc.gpsimd.dma_start(
                            hj[:], cc_out.ap()[j * P:(j + 1) * P,
                                               half * W:(half + 1) * W])
                        m = accp.tile([P, W], BF16, tag="hm",
                                      name=f"hm_{half}_{j}")
                        nc.vector.tensor_scalar_mul(
                            out=m[:], in0=hj[:], scalar1=sel_sb[:, j:j + 1])
                        if acc is None:
                            acc = m
                        elif j < 3:
                            nxt = accp.tile([P, W], BF16, tag="ha",
                                            name=f"ha_{half}_{j}")
                            nc.vector.tensor_add(out=nxt[:], in0=acc[:],
                                                 in1=m[:])
                            acc = nxt
                        else:
                            nc.vector.tensor_add(
                                out=dst[:].rearrange("p a b -> p (a b)"),
                                in0=acc[:], in1=m[:])

            # ---- attention: blocks of (kv-group, 128-query sub-chunk) -------
            # 512 matmul columns = 4 heads x 128 queries; 5 key tiles each.
            # qs descending + r descending puts own-chunk key tiles first so
            # the attention pipeline starts before the halo select lands.
            o_sb = res.tile([P, KV, 4, 4, P], BF16, tag="o")
            for qs in (3, 2, 1, 0):
                for kvb in range(KV):
                    av_ps = ps2.tile([P, CH], F32, tag="av")
                    dn_ps = psd.tile([1, CH], F32, tag="dn")
                    ptsum = None
                    for ri, r in enumerate((4, 3, 2, 1, 0)):
                        jt = qs + r
                        scpool, sctag = (ps1, "sc") if ri % 2 else (pj, "pj")
                        sc_ps = scpool.tile([P, CH], F32, tag=sctag,
                                            name=f"sc_{kvb}_{qs}_{r}")
                        nc.tensor.matmul(sc_ps[:], k_tile(kvb, jt),
                                         q_sb[:, kvb, qs, :, :],
                                         start=True, stop=True)
                        pe = tmp.tile([P, CH], BF16, tag="pe")
                        nc.scalar.activation(
                            out=pe[:], in_=sc_ps[:],
                            func=mybir.ActivationFunctionType.Exp,
                            scale=SCALE)
                        pt = tmp.tile([P, CH], BF16, tag="pt",
                                      name=f"pt_{kvb}_{qs}_{r}")
                        nc.vector.tensor_mul(out=pt[:], in0=pe[:],
                                             in1=mask_sb[:, qs * 5 + r, :])
                        nc.tensor.matmul(av_ps[:], v_tile(kvb, jt), pt[:],
                                         start=(ri == 0), stop=(ri == 4))
                        if ri == 0:
                            ptsum = pt
                        else:
                            nxt = accp.tile([P, CH], BF16, tag="pts",
                                            name=f"pts_{kvb}_{qs}_{r}")
                            nc.vector.tensor_add(out=nxt[:], in0=ptsum[:],
                                                 in1=pt[:])
                            ptsum = nxt
                    nc.tensor.matmul(dn_ps[:], ones_sb[:], ptsum[:],
                                     start=True, stop=True)
                    den = tmp.tile([1, CH], F32, tag="den")
                    nc.vector.tensor_copy(out=den[:], in_=dn_ps[:])
                    bc = tmp.tile([P, CH], F32, tag="bc")
                    nc.gpsimd.partition_broadcast(bc[:], den[:])
                    rc = tmp.tile([P, CH], F32, tag="rc")
                    nc.vector.reciprocal_approx_fast(out=rc[:], in_=bc[:])
                    nc.vector.tensor_mul(
                        out=o_sb[:, kvb, qs, :, :].rearrange("p a b -> p (a b)"),
                        in0=av_ps[:], in1=rc[:])

            # ------------- out projection, transposed: yt = sum_f woT @ oT ---
            wo3 = wo_all.ap().rearrange("h p c -> p h c")
            for et in range(ECH):
                wo_t = wop.tile([P, H, P], BF16, tag="wo", name=f"wo_{et}")
                nc.sync.dma_start(
                    wo_t[:],
                    wo3[:, et, :].rearrange("p (fo c) -> p fo c", fo=H))
                y_ps = pj.tile([P, CH], F32, tag="pj")
                for f in range(H):
                    nc.tensor.matmul(y_ps[:], wo_t[:, f, :],
                                     o_sb[:, f // 4, :, f % 4, :],
                                     start=(f == 0), stop=(f == H - 1))
                y_sb = tmp.tile([P, CH], BF16, tag="ysb")
                nc.vector.tensor_copy(out=y_sb[:], in_=y_ps[:])
                nc.sync.dma_start(yt.ap()[et * P:(et + 1) * P, :], y_sb[:])

    nc.compile()
    return nc


def _host_constants():
    import ml_dtypes
    inv_freq = (1.0 / (THETA ** (np.arange(0, D, 2, dtype=np.float32) / D))
                ).astype(np.float32)
    ang = np.arange(S, dtype=np.float32)[:, None] * inv_freq[None, :]
    emb = np.concatenate([ang, ang], axis=-1)          # [S, D]
    cos_t = np.ascontiguousarray(np.cos(emb).astype(np.float32).T)  # [D, S]
    sin_t = np.ascontiguousarray(np.sin(emb).astype(np.float32).T)

    pm = np.zeros((P, P), dtype=np.float32)            # rotate-half as lhsT
    a = np.arange(64)
    pm[a, a + 64] = 1.0
    pm[a + 64, a] = -1.0
    pm = pm.astype(ml_dtypes.bfloat16)

    onesv = np.ones((1, P), dtype=ml_dtypes.bfloat16)
    return cos_t, sin_t, pm, onesv


def _masks_for_chunk(chunk):
    """[20, 128, 512] bf16: mask[qs*5+r, j, :] for (qsub, r) blocks.

    Columns are 4 heads x 128 queries of sub-chunk qs; the mask depends only
    on the query position, so the four 128-col groups are equal."""
    import ml_dtypes
    m = np.zeros((20, P, CH), dtype=np.float32)
    s0 = chunk * CH
    for qs in range(4):
        qg = s0 + qs * P + np.arange(P)[None, :]       # [1, 128] query pos
        for r in range(5):
            jt = qs + r
            jg = s0 - WIN + jt * P + np.arange(P)[:, None]  # [128, 1] key pos
            dlt = qg - jg
            ok = ((dlt >= 0) & (dlt < WIN) & (jg >= 0)).astype(np.float32)
            m[qs * 5 + r] = np.tile(ok, (1, 4))
    return m.astype(ml_dtypes.bfloat16)


def _pack_pe(w, ncols):
    """[E, ncols] f32 -> [128, (E/128)*ncols] bf16, partition-contiguous."""
    import ml_dtypes
    return np.ascontiguousarray(
        w.reshape(ECH, P, ncols).transpose(1, 0, 2).reshape(P, ECH * ncols)
    ).astype(ml_dtypes.bfloat16)


def _pack_pe_x(xt_sl):
    """[E, 512] f32 -> [128, 16*512] bf16, partition-contiguous."""
    import ml_dtypes
    return np.ascontiguousarray(
        xt_sl.reshape(ECH, P, CH).transpose(1, 0, 2).reshape(P, ECH * CH)
    ).astype(ml_dtypes.bfloat16)


def _prepare_in_maps(x, w_qkv, w_o):
    import ml_dtypes
    cos_t, sin_t, pm, onesv = _host_constants()
    w_qkv = np.asarray(w_qkv, dtype=np.float32)
    w_o = np.asarray(w_o, dtype=np.float32)

    wq_all = np.stack([_pack_pe(w_qkv[:, f * P:(f + 1) * P], P)
                       for f in range(H)])             # [16, 128, 2048]
    KOFF = H * D
    VOFF = H * D + KV * D
    wk_all = np.stack([_pack_pe(w_qkv[:, KOFF + f * P:KOFF + (f + 1) * P], P)
                       for f in range(KV)])            # [4, 128, 2048]
    wv_all = _pack_pe(w_qkv[:, VOFF:VOFF + KV * D], KV * D)   # [128, 8192]
    wo_all = np.stack([_pack_pe(w_o[:, e * P:(e + 1) * P], P)
                       for e in range(ECH)])           # [16, 128, 2048]

    in_maps = []
    xts = [np.ascontiguousarray(np.asarray(x[b], dtype=np.float32).T)
           for b in range(B)]                          # [E, S] f32
    for c in range(NCORES):
        b, chunk = divmod(c, 4)
        s0 = chunk * CH
        im = {
            "xt_own": _pack_pe_x(xts[b][:, s0:s0 + CH]),
            "wq_all": wq_all,
            "wk_all": wk_all,
            "wv_all": wv_all,
            "wo_all": wo_all,
            "cosc": np.ascontiguousarray(cos_t[:, s0:s0 + CH]),
            "sinc": np.ascontiguousarray(sin_t[:, s0:s0 + CH]),
            "masks": _masks_for_chunk(chunk),
            "perm": pm,
            "ones": onesv,
        }
        if USE_HALO_EXCHANGE:
            selv = np.zeros((P, 4), dtype=np.float32)
            if chunk > 0:
                selv[:, chunk - 1] = 1.0
            im["sel"] = selv
        else:
            lo = s0 - WIN
            xh = np.zeros((E, CH), dtype=np.float32)
            ch_ = np.zeros((P, CH), dtype=np.float32)
            sh_ = np.zeros((P, CH), dtype=np.float32)
            if lo >= 0:
                xh[:] = xts[b][:, lo:s0]
                ch_[:] = cos_t[:, lo:s0]
                sh_[:] = sin_t[:, lo:s0]
            im["xt_halo"] = _pack_pe_x(xh)
            im["cosh"] = ch_
            im["sinh"] = sh_
        in_maps.append(im)
    return in_maps


def _install_ntff_shim():
    """bass_utils wants antenv.axon_hooks for trace=True under axon; this
    environment lacks that module, so synthesize it from the boot helper."""
    import sys
    import types
    if "antenv.axon_hooks" in sys.modules:
        return
    try:
        from trn_agent_boot.trn_boot import _ntff_profile_via_ctypes
        hook = _ntff_profile_via_ctypes("/opt/axon/libaxon_pjrt.so")
    except Exception:
        hook = None
    mod = types.ModuleType("antenv.axon_hooks")
    mod.get_axon_ntff_profile_hook = lambda: hook
    mod.set_axon_ntff_profile_hook = lambda h: None
    sys.modules["antenv.axon_hooks"] = mod


def run(x, w_qkv, w_o, trace=False):
    if "nc" not in _CACHE:
        _CACHE["nc"] = _build()
    nc = _CACHE["nc"]
    in_maps = _prepare_in_maps(np.asarray(x), np.asarray(w_qkv),
                               np.asarray(w_o))
    if trace:
        _install_ntff_shim()
    try:
        res = run_bass_kernel_spmd(nc, in_maps, list(range(NCORES)),
                                   trace=trace)
    except Exception:
        if not trace:
            raise
        res = run_bass_kernel_spmd(nc, in_maps, list(range(NCORES)),
                                   trace=False)
    y = np.empty((B, S, E), dtype=np.float32)
    for c in range(NCORES):
        b, chunk = divmod(c, 4)
        y[b, chunk * CH:(chunk + 1) * CH, :] = \
            res.results[c]["yt"].astype(np.float32).T
    return y, res


def kernel(x, w_qkv, w_o):
    y, _ = run(x, w_qkv, w_o, trace=False)
    return y
